# revision 22
# baseline (speedup 1.0000x reference)
"""GAT-style attention score kernel for 8 TRN2 NeuronCores (v3).

Computes out[i,j] = LeakyReLU(Wh[i]@a1 + Wh[j]@a2, slope=0.2) for
N=8192, D=64 -> [8192, 8192] f32 output. Memory-regime problem: the
output write is the wall.

Key decisions, each validated on this hardware:
 - s1/s2 are tiny host matvecs (same spirit as the original kernel's
   host-side transpose/tile/cast prep). Device inputs: s2b =
   tile(s2,(128,1)) f16 2MB; s1f [128,8] f32 (s1f[p,k] = s1[k*128+p]).
 - The device computes and writes the output in F16 and the host
   upcasts: rel-err gate is 2e-2, f16 quantization costs 2.6e-4 total.
   Halves HBM write traffic -> wall drops from ~75us to ~40us.
 - Scalar/ACT lane (cols 0:4800), one op per column:
     out = Prelu(s2b*1 + s1f[:,k], alpha=0.2)   [alpha honored EXACTLY
   on HW; Lrelu's table slope is hardwired 0.01. First act after reset
   computes with garbage scale/bias state -> junk warmups + gap.]
 - Vector/DVE + Pool/GpSimd lanes (cols 4800:8192):
     t = (s2b + s1f[:,k]) * 0.2   [pool: cols 4800:6400,
                                    vector: cols 6400:8192]
     out = (s2b + s1f[:,k]) max t [vector stt, whole 4800:8192]
   (DVE cannot read PSUM at runtime - crashes; gpsimd has no stt and
   no PSUM access; so everything stays in SBUF and pool contributes
   the one op class it has: tensor_scalar with two scalar slots.)
 - DMA layout lessons (measured): one output HWDGE queue only -
   splitting pieces across queues collapses throughput; packets are
   per-partition lines, >=8KB lines run at ~26GB/s/engine, tiny pieces
   tank the stream; inputs ride the sync queue (first chunk - it is
   ready earliest and pre-spins the output path) + pool queue (rest).
 - 3-tile out ring; tile 0 enters and tile 7 exits in small pieces;
   every DMA piece waits only its producer lane's counter; ring reuse
   waits full per-tile DMA semaphores (partial thresholds on a sem
   shared by concurrent DMAs are unsafe - mixed increments).
"""

import os
from contextlib import ExitStack

import numpy as np
import concourse.bass as bass
import concourse.mybir as mybir
from concourse.bass_utils import run_bass_kernel_spmd

N = 8192          # nodes
D = 64            # feature dim
M = 8             # cores
ROWS = N // M     # 1024 output rows per core
NT = ROWS // 128  # 8 row tiles of 128 partitions
SA = 4928         # scalar lane columns   [0:SA)
PM = 6560         # pool does t for [SA:PM), vector for [PM:N)
SB = N - SA       # vector stt columns    [SA:N) = 3392
NEG_SLOPE = 0.2
POOLTS = os.environ.get("POOLTS", "0") == "1"

# input chunks: 0 on sync queue, 1-4 on pool queue
CHUNKS = [(0, 1024), (1024, 2944), (2944, SA), (SA, PM), (PM, N)]

SACTS0 = [(0, 1024), (1024, 2944), (2944, SA)]
SACTS = [(0, SA)]

_cache = {}


def _build():
    nc = bass.Bass()
    f32 = mybir.dt.float32
    f16 = mybir.dt.float16

    s2b_ext = nc.declare_dram_parameter("s2b", [128, N], f16, isOutput=False)
    s1f_ext = nc.declare_dram_parameter("s1f", [128, NT], f32, isOutput=False)
    out_ext = nc.declare_dram_parameter("out", [ROWS, N], f16, isOutput=True)

    with ExitStack() as ctx:
        sb_s2b = ctx.enter_context(nc.sbuf_tensor("sb_s2b", [128, N], f16))
        sb_s1f = ctx.enter_context(nc.sbuf_tensor("sb_s1f", [128, NT], f32))
        sb_junk = ctx.enter_context(nc.sbuf_tensor("sb_junk", [128, 1], f32))
        sb_t0 = ctx.enter_context(nc.sbuf_tensor("sb_t0", [128, SB], f16))
        sb_t1 = ctx.enter_context(nc.sbuf_tensor("sb_t1", [128, SB], f16))
        sb_o0 = ctx.enter_context(nc.sbuf_tensor("sb_o0", [128, N], f16))
        sb_o1 = ctx.enter_context(nc.sbuf_tensor("sb_o1", [128, N], f16))
        sb_o2 = ctx.enter_context(nc.sbuf_tensor("sb_o2", [128, N], f16))
        sb_o3 = ctx.enter_context(nc.sbuf_tensor("sb_o3", [128, N], f16))
        dch = [ctx.enter_context(nc.semaphore(f"dch{c}")) for c in range(5)]
        ds1 = ctx.enter_context(nc.semaphore("ds1"))
        ssem = ctx.enter_context(nc.semaphore("ssem"))
        pg = ctx.enter_context(nc.semaphore("pg"))
        vg = ctx.enter_context(nc.semaphore("vg"))
        vsem = ctx.enter_context(nc.semaphore("vsem"))
        tdsS = [ctx.enter_context(nc.semaphore(f"ts{k}")) for k in range(NT)]
        tdsV = [ctx.enter_context(nc.semaphore(f"tv{k}")) for k in range(NT)]
        block = ctx.enter_context(nc.Block())
        sb_o = [sb_o0, sb_o1, sb_o2, sb_o3]
        RING = 4
        sb_t = [sb_t0, sb_t1]

        # piece plan: (lo, hi, lane, lane_threshold); lane s->ssem, v->vsem
        sc = 0
        vc = 0
        piece_plan = []
        for k in range(NT):
            acts = SACTS0 if k == 0 else SACTS
            pieces = []
            for lo, hi in acts:
                sc += 1
                pieces.append((lo, hi, "s", sc))
            if k in (0, NT - 1):
                vc += 1
                pieces.append((SA, PM, "v", vc))
                vc += 1
                pieces.append((PM, N, "v", vc))
            else:
                vc += 1
                pieces.append((SA, N, "v", vc))
            piece_plan.append(pieces)

        # vsem value after tile k's stt(s) fully done
        VSEM_T = []
        _v = 0
        for k in range(NT):
            _v += 2 if k in (0, NT - 1) else 1
            VSEM_T.append(_v)

        NS = [sum(1 for p in piece_plan[k] if p[2] == "s") for k in range(NT)]
        NV = [sum(1 for p in piece_plan[k] if p[2] == "v") for k in range(NT)]

        flat = [(k, i) for k in range(NT) for i in range(len(piece_plan[k]))]
        # tile0's last v-piece emitted after tile1's s-piece (tile0 ring
        # reuse is far off; avoids stalling the queue on the coldest lane)
        v0b = (0, len(piece_plan[0]) - 1)
        flat.remove(v0b)
        flat.insert(flat.index((1, 0)) + 1, v0b)

        @block.gpsimd
        def _(pool):
            for c in (1, 2, 3, 4):
                lo, hi = CHUNKS[c]
                pool.dma_start(
                    sb_s2b[:, lo:hi], s2b_ext[:, lo:hi]
                ).then_inc(dch[c], 16)
            if POOLTS:
                pool.wait_ge(ds1, 16)
                pool.wait_ge(dch[3], 16)
                for k in range(NT):
                    if k >= 2:
                        pool.wait_ge(vsem, VSEM_T[k - 2])  # t buf reusable
                    pool.tensor_scalar(
                        sb_t[k % 2][:, 0:PM - SA], sb_s2b[:, SA:PM],
                        sb_s1f[:, k:k + 1], NEG_SLOPE,
                        mybir.AluOpType.add, mybir.AluOpType.mult,
                    ).then_inc(pg)

        @block.scalar
        def _(scalar):
            # warm the act path: first activation after reset computes with
            # garbage scale/bias state; the input wait provides the gap
            for _ in range(2):
                scalar.activation(
                    sb_junk[:, :], sb_junk[:, :],
                    mybir.ActivationFunctionType.Prelu,
                    bias=sb_junk[:, 0:1], scale=1.0, alpha=NEG_SLOPE,
                )
            scalar.wait_ge(ds1, 16)
            for k in range(NT):
                acts = SACTS0 if k == 0 else SACTS
                for j, (lo, hi) in enumerate(acts):
                    if k == 0:
                        need = next(c for c, (_, ch) in enumerate(CHUNKS)
                                    if ch >= hi)
                        scalar.wait_ge(dch[need], 16)
                    elif k == 1 and j == 0:
                        scalar.wait_ge(dch[2], 16)
                    if k >= RING and j == 0:
                        scalar.wait_ge(tdsS[k - RING], 16 * NS[k - RING])
                    scalar.activation(
                        sb_o[k % RING][:, lo:hi], sb_s2b[:, lo:hi],
                        mybir.ActivationFunctionType.Prelu,
                        bias=sb_s1f[:, k:k + 1], scale=1.0, alpha=NEG_SLOPE,
                    ).then_inc(ssem)

        @block.vector
        def _(vector):
            vector.wait_ge(ds1, 16)
            ng = 0
            for k in range(NT):
                if k >= RING:
                    vector.wait_ge(tdsV[k - RING], 16 * NV[k - RING])
                if k == 0:
                    vector.wait_ge(dch[4], 16)
                # t for [PM:N) (pool covers [SA:PM) when POOLTS)
                tlo = PM if POOLTS else SA
                vector.tensor_scalar(
                    sb_t[k % 2][:, tlo - SA:SB], sb_s2b[:, tlo:N],
                    sb_s1f[:, k:k + 1], NEG_SLOPE,
                    mybir.AluOpType.add, mybir.AluOpType.mult,
                ).then_inc(vg)
                ng += 1
                vector.wait_ge(vg, ng)          # own-engine RAW guard on t
                if POOLTS:
                    vector.wait_ge(pg, k + 1)   # pool's t half ready
                if k == 0:
                    vector.wait_ge(dch[3], 16)
                halves = ((SA, PM), (PM, N)) if k in (0, NT - 1) else ((SA, N),)
                for lo, hi in halves:
                    vector.scalar_tensor_tensor(
                        sb_o[k % RING][:, lo:hi], sb_s2b[:, lo:hi],
                        sb_s1f[:, k:k + 1], sb_t[k % 2][:, lo - SA:hi - SA],
                        mybir.AluOpType.add, mybir.AluOpType.max,
                    ).then_inc(vsem)

        @block.sync
        def _(sync):
            # s1f + first s2b chunk ride the sync queue: ready earliest,
            # and this pre-spins the output HWDGE path
            sync.dma_start(sb_s1f[:, :], s1f_ext[:, :]).then_inc(ds1, 16)
            lo, hi = CHUNKS[0]
            sync.dma_start(sb_s2b[:, lo:hi], s2b_ext[:, lo:hi]).then_inc(dch[0], 16)
            for (k, i) in flat:
                lo, hi, lane, thr = piece_plan[k][i]
                sync.wait_ge(ssem if lane == "s" else vsem, thr)
                sync.dma_start(
                    out_ext[k * 128:(k + 1) * 128, lo:hi],
                    sb_o[k % RING][:, lo:hi],
                ).then_inc((tdsS if lane == "s" else tdsV)[k], 16)

    return nc


def _run(Wh, a, trace=False, **kw):
    Wh = np.ascontiguousarray(np.asarray(Wh, dtype=np.float32))
    a = np.ascontiguousarray(np.asarray(a, dtype=np.float32))
    assert Wh.shape == (N, D) and a.shape == (2 * D, 1)

    if "nc" not in _cache:
        _cache["nc"] = _build()
    nc = _cache["nc"]

    a1 = a[:D, 0]
    a2 = a[D:, 0]
    s1 = Wh @ a1                      # [N]
    s2 = Wh @ a2                      # [N]
    s2b = np.ascontiguousarray(
        np.broadcast_to(s2.astype(np.float16)[None, :], (128, N)))
    in_maps = []
    for i in range(M):
        s1i = s1[i * ROWS:(i + 1) * ROWS].astype(np.float32)
        s1f = np.ascontiguousarray(s1i.reshape(NT, 128).T)  # [128, NT]
        in_maps.append({"s2b": s2b, "s1f": s1f})
    res = run_bass_kernel_spmd(nc, in_maps, core_ids=list(range(M)), trace=trace, **kw)
    out = np.concatenate(
        [res.results[i]["out"].astype(np.float32) for i in range(M)], axis=0)
    return out, res


def kernel(Wh, a):
    return _run(Wh, a)[0]


# revision 23
# speedup vs baseline: 1.1322x; 1.1322x over previous
"""GAT-style attention score kernel for 8 TRN2 NeuronCores (v3).

Computes out[i,j] = LeakyReLU(Wh[i]@a1 + Wh[j]@a2, slope=0.2) for
N=8192, D=64 -> [8192, 8192] f32 output. Memory-regime problem: the
output write is the wall.

Key decisions, each validated on this hardware:
 - s1/s2 are tiny host matvecs (same spirit as the original kernel's
   host-side transpose/tile/cast prep). Device inputs: s2b =
   tile(s2,(128,1)) f16 2MB; s1f [128,8] f32 (s1f[p,k] = s1[k*128+p]).
 - The device computes and writes the output in F16 and the host
   upcasts: rel-err gate is 2e-2, f16 quantization costs 2.6e-4 total.
   Halves HBM write traffic -> wall drops from ~75us to ~40us.
 - Scalar/ACT lane (cols 0:4800), one op per column:
     out = Prelu(s2b*1 + s1f[:,k], alpha=0.2)   [alpha honored EXACTLY
   on HW; Lrelu's table slope is hardwired 0.01. First act after reset
   computes with garbage scale/bias state -> junk warmups + gap.]
 - Vector/DVE + Pool/GpSimd lanes (cols 4800:8192):
     t = (s2b + s1f[:,k]) * 0.2   [pool: cols 4800:6400,
                                    vector: cols 6400:8192]
     out = (s2b + s1f[:,k]) max t [vector stt, whole 4800:8192]
   (DVE cannot read PSUM at runtime - crashes; gpsimd has no stt and
   no PSUM access; so everything stays in SBUF and pool contributes
   the one op class it has: tensor_scalar with two scalar slots.)
 - DMA layout lessons (measured): one output HWDGE queue only -
   splitting pieces across queues collapses throughput; packets are
   per-partition lines, >=8KB lines run at ~26GB/s/engine, tiny pieces
   tank the stream; inputs ride the sync queue (first chunk - it is
   ready earliest and pre-spins the output path) + pool queue (rest).
 - 3-tile out ring; tile 0 enters and tile 7 exits in small pieces;
   every DMA piece waits only its producer lane's counter; ring reuse
   waits full per-tile DMA semaphores (partial thresholds on a sem
   shared by concurrent DMAs are unsafe - mixed increments).
"""

import os
from contextlib import ExitStack

import numpy as np
import concourse.bass as bass
import concourse.mybir as mybir
from concourse.bass_utils import run_bass_kernel_spmd

N = 8192          # nodes
D = 64            # feature dim
M = 8             # cores
ROWS = N // M     # 1024 output rows per core
NT = ROWS // 128  # 8 row tiles of 128 partitions
SA = 4800         # scalar lane columns   [0:SA)
PM = 6400         # pool does t for [SA:PM), vector for [PM:N)
SB = N - SA       # vector stt columns    [SA:N) = 3392
NEG_SLOPE = 0.2
POOLTS = os.environ.get("POOLTS", "0") == "1"

# input chunks: 0 on sync queue, 1-4 on pool queue
CHUNKS = [(0, 1024), (1024, 2944), (2944, SA), (SA, PM), (PM, N)]

SACTS0 = [(0, 1024), (1024, 2944), (2944, SA)]
SACTS = [(0, SA)]

_cache = {}


def _build():
    nc = bass.Bass()
    f32 = mybir.dt.float32
    f16 = mybir.dt.float16

    s2b_ext = nc.declare_dram_parameter("s2b", [128, N], f16, isOutput=False)
    s1f_ext = nc.declare_dram_parameter("s1f", [128, NT], f32, isOutput=False)
    out_ext = nc.declare_dram_parameter("out", [ROWS, N], f16, isOutput=True)

    with ExitStack() as ctx:
        sb_s2b = ctx.enter_context(nc.sbuf_tensor("sb_s2b", [128, N], f16))
        sb_s1f = ctx.enter_context(nc.sbuf_tensor("sb_s1f", [128, NT], f32))
        sb_junk = ctx.enter_context(nc.sbuf_tensor("sb_junk", [128, 1], f32))
        sb_t0 = ctx.enter_context(nc.sbuf_tensor("sb_t0", [128, SB], f16))
        sb_t1 = ctx.enter_context(nc.sbuf_tensor("sb_t1", [128, SB], f16))
        sb_o0 = ctx.enter_context(nc.sbuf_tensor("sb_o0", [128, N], f16))
        sb_o1 = ctx.enter_context(nc.sbuf_tensor("sb_o1", [128, N], f16))
        sb_o2 = ctx.enter_context(nc.sbuf_tensor("sb_o2", [128, N], f16))
        sb_o3 = ctx.enter_context(nc.sbuf_tensor("sb_o3", [128, N], f16))
        dch = [ctx.enter_context(nc.semaphore(f"dch{c}")) for c in range(5)]
        ds1 = ctx.enter_context(nc.semaphore("ds1"))
        ssem = ctx.enter_context(nc.semaphore("ssem"))
        pg = ctx.enter_context(nc.semaphore("pg"))
        vg = ctx.enter_context(nc.semaphore("vg"))
        vsem = ctx.enter_context(nc.semaphore("vsem"))
        tdsS = [ctx.enter_context(nc.semaphore(f"ts{k}")) for k in range(NT)]
        tdsV = [ctx.enter_context(nc.semaphore(f"tv{k}")) for k in range(NT)]
        block = ctx.enter_context(nc.Block())
        sb_o = [sb_o0, sb_o1, sb_o2, sb_o3]
        RING = 4
        sb_t = [sb_t0, sb_t1]

        # piece plan: (lo, hi, lane, lane_threshold); lane s->ssem, v->vsem
        sc = 0
        vc = 0
        piece_plan = []
        for k in range(NT):
            acts = SACTS0 if k == 0 else SACTS
            pieces = []
            for lo, hi in acts:
                sc += 1
                pieces.append((lo, hi, "s", sc))
            if k in (0, NT - 1):
                vc += 1
                pieces.append((SA, PM, "v", vc))
                vc += 1
                pieces.append((PM, N, "v", vc))
            else:
                vc += 1
                pieces.append((SA, N, "v", vc))
            piece_plan.append(pieces)

        # vsem value after tile k's stt(s) fully done
        VSEM_T = []
        _v = 0
        for k in range(NT):
            _v += 2 if k in (0, NT - 1) else 1
            VSEM_T.append(_v)

        NS = [sum(1 for p in piece_plan[k] if p[2] == "s") for k in range(NT)]
        NV = [sum(1 for p in piece_plan[k] if p[2] == "v") for k in range(NT)]

        flat = [(k, i) for k in range(NT) for i in range(len(piece_plan[k]))]
        # tile0's last v-piece emitted after tile1's s-piece (tile0 ring
        # reuse is far off; avoids stalling the queue on the coldest lane)
        v0b = (0, len(piece_plan[0]) - 1)
        flat.remove(v0b)
        flat.insert(flat.index((1, 0)) + 1, v0b)

        @block.gpsimd
        def _(pool):
            for c in (1, 2, 3, 4):
                lo, hi = CHUNKS[c]
                pool.dma_start(
                    sb_s2b[:, lo:hi], s2b_ext[:, lo:hi]
                ).then_inc(dch[c], 16)
            if POOLTS:
                pool.wait_ge(ds1, 16)
                pool.wait_ge(dch[3], 16)
                for k in range(NT):
                    if k >= 2:
                        pool.wait_ge(vsem, VSEM_T[k - 2])  # t buf reusable
                    pool.tensor_scalar(
                        sb_t[k % 2][:, 0:PM - SA], sb_s2b[:, SA:PM],
                        sb_s1f[:, k:k + 1], NEG_SLOPE,
                        mybir.AluOpType.add, mybir.AluOpType.mult,
                    ).then_inc(pg)

        @block.scalar
        def _(scalar):
            # warm the act path: first activation after reset computes with
            # garbage scale/bias state; the input wait provides the gap
            for _ in range(2):
                scalar.activation(
                    sb_junk[:, :], sb_junk[:, :],
                    mybir.ActivationFunctionType.Prelu,
                    bias=sb_junk[:, 0:1], scale=1.0, alpha=NEG_SLOPE,
                )
            scalar.wait_ge(ds1, 16)
            for k in range(NT):
                acts = SACTS0 if k == 0 else SACTS
                for j, (lo, hi) in enumerate(acts):
                    if k == 0:
                        need = next(c for c, (_, ch) in enumerate(CHUNKS)
                                    if ch >= hi)
                        scalar.wait_ge(dch[need], 16)
                    elif k == 1 and j == 0:
                        scalar.wait_ge(dch[2], 16)
                    if k >= RING and j == 0:
                        scalar.wait_ge(tdsS[k - RING], 16 * NS[k - RING])
                    scalar.activation(
                        sb_o[k % RING][:, lo:hi], sb_s2b[:, lo:hi],
                        mybir.ActivationFunctionType.Prelu,
                        bias=sb_s1f[:, k:k + 1], scale=1.0, alpha=NEG_SLOPE,
                    ).then_inc(ssem)

        @block.vector
        def _(vector):
            vector.wait_ge(ds1, 16)
            ng = 0
            for k in range(NT):
                if k >= RING:
                    vector.wait_ge(tdsV[k - RING], 16 * NV[k - RING])
                if k == 0:
                    vector.wait_ge(dch[4], 16)
                # t for [PM:N) (pool covers [SA:PM) when POOLTS)
                tlo = PM if POOLTS else SA
                vector.tensor_scalar(
                    sb_t[k % 2][:, tlo - SA:SB], sb_s2b[:, tlo:N],
                    sb_s1f[:, k:k + 1], NEG_SLOPE,
                    mybir.AluOpType.add, mybir.AluOpType.mult,
                ).then_inc(vg)
                ng += 1
                vector.wait_ge(vg, ng)          # own-engine RAW guard on t
                if POOLTS:
                    vector.wait_ge(pg, k + 1)   # pool's t half ready
                if k == 0:
                    vector.wait_ge(dch[3], 16)
                halves = ((SA, PM), (PM, N)) if k in (0, NT - 1) else ((SA, N),)
                for lo, hi in halves:
                    vector.scalar_tensor_tensor(
                        sb_o[k % RING][:, lo:hi], sb_s2b[:, lo:hi],
                        sb_s1f[:, k:k + 1], sb_t[k % 2][:, lo - SA:hi - SA],
                        mybir.AluOpType.add, mybir.AluOpType.max,
                    ).then_inc(vsem)

        @block.sync
        def _(sync):
            # s1f + first s2b chunk ride the sync queue: ready earliest,
            # and this pre-spins the output HWDGE path
            sync.dma_start(sb_s1f[:, :], s1f_ext[:, :]).then_inc(ds1, 16)
            lo, hi = CHUNKS[0]
            sync.dma_start(sb_s2b[:, lo:hi], s2b_ext[:, lo:hi]).then_inc(dch[0], 16)
            for (k, i) in flat:
                lo, hi, lane, thr = piece_plan[k][i]
                sync.wait_ge(ssem if lane == "s" else vsem, thr)
                sync.dma_start(
                    out_ext[k * 128:(k + 1) * 128, lo:hi],
                    sb_o[k % RING][:, lo:hi],
                ).then_inc((tdsS if lane == "s" else tdsV)[k], 16)

    return nc


def _run(Wh, a, trace=False, **kw):
    Wh = np.ascontiguousarray(np.asarray(Wh, dtype=np.float32))
    a = np.ascontiguousarray(np.asarray(a, dtype=np.float32))
    assert Wh.shape == (N, D) and a.shape == (2 * D, 1)

    if "nc" not in _cache:
        _cache["nc"] = _build()
    nc = _cache["nc"]

    a1 = a[:D, 0]
    a2 = a[D:, 0]
    s1 = Wh @ a1                      # [N]
    s2 = Wh @ a2                      # [N]
    s2b = np.ascontiguousarray(
        np.broadcast_to(s2.astype(np.float16)[None, :], (128, N)))
    in_maps = []
    for i in range(M):
        s1i = s1[i * ROWS:(i + 1) * ROWS].astype(np.float32)
        s1f = np.ascontiguousarray(s1i.reshape(NT, 128).T)  # [128, NT]
        in_maps.append({"s2b": s2b, "s1f": s1f})
    res = run_bass_kernel_spmd(nc, in_maps, core_ids=list(range(M)), trace=trace, **kw)
    out = np.concatenate(
        [res.results[i]["out"].astype(np.float32) for i in range(M)], axis=0)
    return out, res


def kernel(Wh, a):
    return _run(Wh, a)[0]


# revision 24
# speedup vs baseline: 1.1337x; 1.0013x over previous
"""GAT-style attention score kernel for 8 TRN2 NeuronCores (v3).

Computes out[i,j] = LeakyReLU(Wh[i]@a1 + Wh[j]@a2, slope=0.2) for
N=8192, D=64 -> [8192, 8192] f32 output. Memory-regime problem: the
output write is the wall.

Key decisions, each validated on this hardware:
 - s1/s2 are tiny host matvecs (same spirit as the original kernel's
   host-side transpose/tile/cast prep). Device inputs: s2b =
   tile(s2,(128,1)) f16 2MB; s1f [128,8] f32 (s1f[p,k] = s1[k*128+p]).
 - The device computes and writes the output in F16 and the host
   upcasts: rel-err gate is 2e-2, f16 quantization costs 2.6e-4 total.
   Halves HBM write traffic -> wall drops from ~75us to ~40us.
 - Scalar/ACT lane (cols 0:4800), one op per column:
     out = Prelu(s2b*1 + s1f[:,k], alpha=0.2)   [alpha honored EXACTLY
   on HW; Lrelu's table slope is hardwired 0.01. First act after reset
   computes with garbage scale/bias state -> junk warmups + gap.]
 - Vector/DVE lane (cols 4800:8192), two ops per column:
     t = (s2b + s1f[:,k]) * 0.2;  out = (s2b + s1f[:,k]) max t
   (DVE cannot read PSUM at runtime - crashes; gpsimd has no stt and
   no PSUM access; so everything stays in SBUF. POOLTS=1 offloads part
   of the t-pass to the pool engine via tensor_scalar - measured
   SLOWER: pool's ts lags and the stt's pg wait stalls the stream.)
 - DMA layout lessons (measured): one output HWDGE queue only -
   splitting pieces across queues collapses throughput; packets are
   per-partition lines, >=8KB lines run at ~26GB/s/engine, tiny pieces
   tank the stream; inputs ride the sync queue (first chunk - it is
   ready earliest and pre-spins the output path) + pool queue (rest).
 - 4-tile out ring with PER-LANE per-tile DMA semaphores: scalar's
   ring reuse waits only its own columns' pieces, so the faster lane
   never stalls on the slower one. (Partial thresholds on a semaphore
   shared by concurrent DMAs are unsafe - mixed increments - hence
   separate sems, each waited at its full total.)
 - Startup: tile 0 enters in 3 scalar + 2 vector pieces; tile 7 exits
   in small pieces (tail). Output pieces are emitted strictly in tile
   order (the queue is in-order; readiness-reordering measured slower),
   except tile0's last vector piece which trails tile1's first piece.
"""

import os
from contextlib import ExitStack

import numpy as np
import concourse.bass as bass
import concourse.mybir as mybir
from concourse.bass_utils import run_bass_kernel_spmd

N = 8192          # nodes
D = 64            # feature dim
M = 8             # cores
ROWS = N // M     # 1024 output rows per core
NT = ROWS // 128  # 8 row tiles of 128 partitions
SA = 4800         # scalar lane columns   [0:SA)
PM = 6400         # pool does t for [SA:PM), vector for [PM:N)
SB = N - SA       # vector stt columns    [SA:N) = 3392
NEG_SLOPE = 0.2
POOLTS = os.environ.get("POOLTS", "0") == "1"

# input chunks: 0 on sync queue, 1-4 on pool queue
CHUNKS = [(0, 1024), (1024, 2944), (2944, SA), (SA, PM), (PM, N)]

SACTS0 = [(0, 1024), (1024, 2944), (2944, SA)]
SACTS = [(0, SA)]

_cache = {}


def _build():
    nc = bass.Bass()
    f32 = mybir.dt.float32
    f16 = mybir.dt.float16

    s2b_ext = nc.declare_dram_parameter("s2b", [128, N], f16, isOutput=False)
    s1f_ext = nc.declare_dram_parameter("s1f", [128, NT], f32, isOutput=False)
    out_ext = nc.declare_dram_parameter("out", [ROWS, N], f16, isOutput=True)

    with ExitStack() as ctx:
        sb_s2b = ctx.enter_context(nc.sbuf_tensor("sb_s2b", [128, N], f16))
        sb_s1f = ctx.enter_context(nc.sbuf_tensor("sb_s1f", [128, NT], f32))
        sb_junk = ctx.enter_context(nc.sbuf_tensor("sb_junk", [128, 1], f32))
        sb_t0 = ctx.enter_context(nc.sbuf_tensor("sb_t0", [128, SB], f16))
        sb_t1 = ctx.enter_context(nc.sbuf_tensor("sb_t1", [128, SB], f16))
        sb_o0 = ctx.enter_context(nc.sbuf_tensor("sb_o0", [128, N], f16))
        sb_o1 = ctx.enter_context(nc.sbuf_tensor("sb_o1", [128, N], f16))
        sb_o2 = ctx.enter_context(nc.sbuf_tensor("sb_o2", [128, N], f16))
        sb_o3 = ctx.enter_context(nc.sbuf_tensor("sb_o3", [128, N], f16))
        dch = [ctx.enter_context(nc.semaphore(f"dch{c}")) for c in range(5)]
        ds1 = ctx.enter_context(nc.semaphore("ds1"))
        ssem = ctx.enter_context(nc.semaphore("ssem"))
        pg = ctx.enter_context(nc.semaphore("pg"))
        vg = ctx.enter_context(nc.semaphore("vg"))
        vsem = ctx.enter_context(nc.semaphore("vsem"))
        tdsS = [ctx.enter_context(nc.semaphore(f"ts{k}")) for k in range(NT)]
        tdsV = [ctx.enter_context(nc.semaphore(f"tv{k}")) for k in range(NT)]
        block = ctx.enter_context(nc.Block())
        sb_o = [sb_o0, sb_o1, sb_o2, sb_o3]
        RING = 4
        sb_t = [sb_t0, sb_t1]

        # piece plan: (lo, hi, lane, lane_threshold); lane s->ssem, v->vsem
        sc = 0
        vc = 0
        piece_plan = []
        for k in range(NT):
            acts = SACTS0 if k == 0 else SACTS
            pieces = []
            for lo, hi in acts:
                sc += 1
                pieces.append((lo, hi, "s", sc))
            if k in (0, NT - 1):
                vc += 1
                pieces.append((SA, PM, "v", vc))
                vc += 1
                pieces.append((PM, N, "v", vc))
            else:
                vc += 1
                pieces.append((SA, N, "v", vc))
            piece_plan.append(pieces)

        # vsem value after tile k's stt(s) fully done
        VSEM_T = []
        _v = 0
        for k in range(NT):
            _v += 2 if k in (0, NT - 1) else 1
            VSEM_T.append(_v)

        NS = [sum(1 for p in piece_plan[k] if p[2] == "s") for k in range(NT)]
        NV = [sum(1 for p in piece_plan[k] if p[2] == "v") for k in range(NT)]

        flat = [(k, i) for k in range(NT) for i in range(len(piece_plan[k]))]
        # tile0's last v-piece emitted after tile1's s-piece (tile0 ring
        # reuse is far off; avoids stalling the queue on the coldest lane)
        v0b = (0, len(piece_plan[0]) - 1)
        flat.remove(v0b)
        flat.insert(flat.index((1, 0)) + 1, v0b)

        @block.gpsimd
        def _(pool):
            for c in (1, 2, 3, 4):
                lo, hi = CHUNKS[c]
                pool.dma_start(
                    sb_s2b[:, lo:hi], s2b_ext[:, lo:hi]
                ).then_inc(dch[c], 16)
            if POOLTS:
                pool.wait_ge(ds1, 16)
                pool.wait_ge(dch[3], 16)
                for k in range(NT):
                    if k >= 2:
                        pool.wait_ge(vsem, VSEM_T[k - 2])  # t buf reusable
                    pool.tensor_scalar(
                        sb_t[k % 2][:, 0:PM - SA], sb_s2b[:, SA:PM],
                        sb_s1f[:, k:k + 1], NEG_SLOPE,
                        mybir.AluOpType.add, mybir.AluOpType.mult,
                    ).then_inc(pg)

        @block.scalar
        def _(scalar):
            # warm the act path: first activation after reset computes with
            # garbage scale/bias state; the input wait provides the gap
            for _ in range(2):
                scalar.activation(
                    sb_junk[:, :], sb_junk[:, :],
                    mybir.ActivationFunctionType.Prelu,
                    bias=sb_junk[:, 0:1], scale=1.0, alpha=NEG_SLOPE,
                )
            scalar.wait_ge(ds1, 16)
            for k in range(NT):
                acts = SACTS0 if k == 0 else SACTS
                for j, (lo, hi) in enumerate(acts):
                    if k == 0:
                        need = next(c for c, (_, ch) in enumerate(CHUNKS)
                                    if ch >= hi)
                        scalar.wait_ge(dch[need], 16)
                    elif k == 1 and j == 0:
                        scalar.wait_ge(dch[2], 16)
                    if k >= RING and j == 0:
                        scalar.wait_ge(tdsS[k - RING], 16 * NS[k - RING])
                    scalar.activation(
                        sb_o[k % RING][:, lo:hi], sb_s2b[:, lo:hi],
                        mybir.ActivationFunctionType.Prelu,
                        bias=sb_s1f[:, k:k + 1], scale=1.0, alpha=NEG_SLOPE,
                    ).then_inc(ssem)

        @block.vector
        def _(vector):
            vector.wait_ge(ds1, 16)
            ng = 0
            for k in range(NT):
                if k >= RING:
                    vector.wait_ge(tdsV[k - RING], 16 * NV[k - RING])
                if k == 0:
                    vector.wait_ge(dch[4], 16)
                # t for [PM:N) (pool covers [SA:PM) when POOLTS)
                tlo = PM if POOLTS else SA
                vector.tensor_scalar(
                    sb_t[k % 2][:, tlo - SA:SB], sb_s2b[:, tlo:N],
                    sb_s1f[:, k:k + 1], NEG_SLOPE,
                    mybir.AluOpType.add, mybir.AluOpType.mult,
                ).then_inc(vg)
                ng += 1
                vector.wait_ge(vg, ng)          # own-engine RAW guard on t
                if POOLTS:
                    vector.wait_ge(pg, k + 1)   # pool's t half ready
                if k == 0:
                    vector.wait_ge(dch[3], 16)
                halves = ((SA, PM), (PM, N)) if k in (0, NT - 1) else ((SA, N),)
                for lo, hi in halves:
                    vector.scalar_tensor_tensor(
                        sb_o[k % RING][:, lo:hi], sb_s2b[:, lo:hi],
                        sb_s1f[:, k:k + 1], sb_t[k % 2][:, lo - SA:hi - SA],
                        mybir.AluOpType.add, mybir.AluOpType.max,
                    ).then_inc(vsem)

        @block.sync
        def _(sync):
            # s1f + first s2b chunk ride the sync queue: ready earliest,
            # and this pre-spins the output HWDGE path
            sync.dma_start(sb_s1f[:, :], s1f_ext[:, :]).then_inc(ds1, 16)
            lo, hi = CHUNKS[0]
            sync.dma_start(sb_s2b[:, lo:hi], s2b_ext[:, lo:hi]).then_inc(dch[0], 16)
            for (k, i) in flat:
                lo, hi, lane, thr = piece_plan[k][i]
                sync.wait_ge(ssem if lane == "s" else vsem, thr)
                sync.dma_start(
                    out_ext[k * 128:(k + 1) * 128, lo:hi],
                    sb_o[k % RING][:, lo:hi],
                ).then_inc((tdsS if lane == "s" else tdsV)[k], 16)

    return nc


def _run(Wh, a, trace=False, **kw):
    Wh = np.ascontiguousarray(np.asarray(Wh, dtype=np.float32))
    a = np.ascontiguousarray(np.asarray(a, dtype=np.float32))
    assert Wh.shape == (N, D) and a.shape == (2 * D, 1)

    if "nc" not in _cache:
        _cache["nc"] = _build()
    nc = _cache["nc"]

    a1 = a[:D, 0]
    a2 = a[D:, 0]
    s1 = Wh @ a1                      # [N]
    s2 = Wh @ a2                      # [N]
    s2b = np.ascontiguousarray(
        np.broadcast_to(s2.astype(np.float16)[None, :], (128, N)))
    in_maps = []
    for i in range(M):
        s1i = s1[i * ROWS:(i + 1) * ROWS].astype(np.float32)
        s1f = np.ascontiguousarray(s1i.reshape(NT, 128).T)  # [128, NT]
        in_maps.append({"s2b": s2b, "s1f": s1f})
    res = run_bass_kernel_spmd(nc, in_maps, core_ids=list(range(M)), trace=trace, **kw)
    out = np.concatenate(
        [res.results[i]["out"].astype(np.float32) for i in range(M)], axis=0)
    return out, res


def kernel(Wh, a):
    return _run(Wh, a)[0]


# revision 25
# speedup vs baseline: 1.1348x; 1.0010x over previous
"""GAT-style attention score kernel for 8 TRN2 NeuronCores (v3).

Computes out[i,j] = LeakyReLU(Wh[i]@a1 + Wh[j]@a2, slope=0.2) for
N=8192, D=64 -> [8192, 8192] f32 output. Memory-regime problem: the
output write is the wall.

Key decisions, each validated on this hardware:
 - s1/s2 are tiny host matvecs (same spirit as the original kernel's
   host-side transpose/tile/cast prep). Device inputs: s2b =
   tile(s2,(128,1)) f16 2MB; s1f [128,8] f32 (s1f[p,k] = s1[k*128+p]).
 - The device computes and writes the output in F16 and the host
   upcasts: rel-err gate is 2e-2, f16 quantization costs 2.6e-4 total.
   Halves HBM write traffic -> wall drops from ~75us to ~40us.
 - Scalar/ACT lane (cols 0:4800), one op per column:
     out = Prelu(s2b*1 + s1f[:,k], alpha=0.2)   [alpha honored EXACTLY
   on HW; Lrelu's table slope is hardwired 0.01. First act after reset
   computes with garbage scale/bias state -> junk warmups + gap.]
 - Vector/DVE lane (cols 4800:8192), two ops per column:
     t = (s2b + s1f[:,k]) * 0.2;  out = (s2b + s1f[:,k]) max t
   (DVE cannot read PSUM at runtime - crashes; gpsimd has no stt and
   no PSUM access; so everything stays in SBUF. POOLTS=1 offloads part
   of the t-pass to the pool engine via tensor_scalar - measured
   SLOWER: pool's ts lags and the stt's pg wait stalls the stream.)
 - DMA layout lessons (measured): one output HWDGE queue only -
   splitting pieces across queues collapses throughput; packets are
   per-partition lines, >=8KB lines run at ~26GB/s/engine, tiny pieces
   tank the stream; inputs ride the sync queue (first chunk - it is
   ready earliest and pre-spins the output path) + pool queue (rest).
 - 4-tile out ring with PER-LANE per-tile DMA semaphores: scalar's
   ring reuse waits only its own columns' pieces, so the faster lane
   never stalls on the slower one. (Partial thresholds on a semaphore
   shared by concurrent DMAs are unsafe - mixed increments - hence
   separate sems, each waited at its full total.)
 - Startup: tile 0 enters in 3 scalar + 2 vector pieces; tile 7 exits
   in small pieces (tail). Output pieces are emitted strictly in tile
   order (the queue is in-order; readiness-reordering measured slower),
   except tile0's last vector piece which trails tile1's first piece.
"""

import os
from contextlib import ExitStack

import numpy as np
import concourse.bass as bass
import concourse.mybir as mybir
from concourse.bass_utils import run_bass_kernel_spmd

N = 8192          # nodes
D = 64            # feature dim
M = 8             # cores
ROWS = N // M     # 1024 output rows per core
NT = ROWS // 128  # 8 row tiles of 128 partitions
SA = 4800         # scalar lane columns   [0:SA)
PM = 6400         # pool does t for [SA:PM), vector for [PM:N)
SB = N - SA       # vector stt columns    [SA:N) = 3392
NEG_SLOPE = 0.2
POOLTS = os.environ.get("POOLTS", "0") == "1"

# input chunks: 0 on sync queue, 1-2 on pool queue (fat lines)
CHUNKS = [(0, 1024), (1024, SA), (SA, N)]

SACTS0 = [(0, 1024), (1024, 2944), (2944, SA)]
SACTS = [(0, SA)]
SACTS_LAST = [(0, 2400), (2400, SA)]  # smaller tail pieces

_cache = {}


def _build():
    nc = bass.Bass()
    f32 = mybir.dt.float32
    f16 = mybir.dt.float16

    s2b_ext = nc.declare_dram_parameter("s2b", [128, N], f16, isOutput=False)
    s1f_ext = nc.declare_dram_parameter("s1f", [128, NT], f32, isOutput=False)
    out_ext = nc.declare_dram_parameter("out", [ROWS, N], f16, isOutput=True)

    with ExitStack() as ctx:
        sb_s2b = ctx.enter_context(nc.sbuf_tensor("sb_s2b", [128, N], f16))
        sb_s1f = ctx.enter_context(nc.sbuf_tensor("sb_s1f", [128, NT], f32))
        sb_junk = ctx.enter_context(nc.sbuf_tensor("sb_junk", [128, 1], f32))
        sb_t0 = ctx.enter_context(nc.sbuf_tensor("sb_t0", [128, SB], f16))
        sb_t1 = ctx.enter_context(nc.sbuf_tensor("sb_t1", [128, SB], f16))
        sb_o0 = ctx.enter_context(nc.sbuf_tensor("sb_o0", [128, N], f16))
        sb_o1 = ctx.enter_context(nc.sbuf_tensor("sb_o1", [128, N], f16))
        sb_o2 = ctx.enter_context(nc.sbuf_tensor("sb_o2", [128, N], f16))
        sb_o3 = ctx.enter_context(nc.sbuf_tensor("sb_o3", [128, N], f16))
        dch = [ctx.enter_context(nc.semaphore(f"dch{c}")) for c in range(3)]
        ds1 = ctx.enter_context(nc.semaphore("ds1"))
        ssem = ctx.enter_context(nc.semaphore("ssem"))
        pg = ctx.enter_context(nc.semaphore("pg"))
        vg = ctx.enter_context(nc.semaphore("vg"))
        vsem = ctx.enter_context(nc.semaphore("vsem"))
        tdsS = [ctx.enter_context(nc.semaphore(f"ts{k}")) for k in range(NT)]
        tdsV = [ctx.enter_context(nc.semaphore(f"tv{k}")) for k in range(NT)]
        block = ctx.enter_context(nc.Block())
        sb_o = [sb_o0, sb_o1, sb_o2, sb_o3]
        RING = 4
        sb_t = [sb_t0, sb_t1]

        # piece plan: (lo, hi, lane, lane_threshold); lane s->ssem, v->vsem
        sc = 0
        vc = 0
        piece_plan = []
        for k in range(NT):
            acts = (SACTS0 if k == 0
                    else SACTS_LAST if k == NT - 1 else SACTS)
            pieces = []
            for lo, hi in acts:
                sc += 1
                pieces.append((lo, hi, "s", sc))
            if k in (0, NT - 1):
                vc += 1
                pieces.append((SA, PM, "v", vc))
                vc += 1
                pieces.append((PM, N, "v", vc))
            else:
                vc += 1
                pieces.append((SA, N, "v", vc))
            piece_plan.append(pieces)

        # vsem value after tile k's stt(s) fully done
        VSEM_T = []
        _v = 0
        for k in range(NT):
            _v += 2 if k in (0, NT - 1) else 1
            VSEM_T.append(_v)

        NS = [sum(1 for p in piece_plan[k] if p[2] == "s") for k in range(NT)]
        NV = [sum(1 for p in piece_plan[k] if p[2] == "v") for k in range(NT)]

        flat = [(k, i) for k in range(NT) for i in range(len(piece_plan[k]))]
        # tile0's last v-piece emitted after tile1's s-piece (tile0 ring
        # reuse is far off; avoids stalling the queue on the coldest lane)
        v0b = (0, len(piece_plan[0]) - 1)
        flat.remove(v0b)
        flat.insert(flat.index((1, 0)) + 1, v0b)

        @block.gpsimd
        def _(pool):
            for c in (1, 2):
                lo, hi = CHUNKS[c]
                pool.dma_start(
                    sb_s2b[:, lo:hi], s2b_ext[:, lo:hi]
                ).then_inc(dch[c], 16)
            if POOLTS:
                pool.wait_ge(ds1, 16)
                pool.wait_ge(dch[2], 16)
                for k in range(NT):
                    if k >= 2:
                        pool.wait_ge(vsem, VSEM_T[k - 2])  # t buf reusable
                    pool.tensor_scalar(
                        sb_t[k % 2][:, 0:PM - SA], sb_s2b[:, SA:PM],
                        sb_s1f[:, k:k + 1], NEG_SLOPE,
                        mybir.AluOpType.add, mybir.AluOpType.mult,
                    ).then_inc(pg)

        @block.scalar
        def _(scalar):
            # warm the act path: first activation after reset computes with
            # garbage scale/bias state; the input wait provides the gap
            for _ in range(2):
                scalar.activation(
                    sb_junk[:, :], sb_junk[:, :],
                    mybir.ActivationFunctionType.Prelu,
                    bias=sb_junk[:, 0:1], scale=1.0, alpha=NEG_SLOPE,
                )
            scalar.wait_ge(ds1, 16)
            for k in range(NT):
                acts = (SACTS0 if k == 0
                        else SACTS_LAST if k == NT - 1 else SACTS)
                for j, (lo, hi) in enumerate(acts):
                    if k == 0 and j == 0:
                        scalar.wait_ge(dch[0], 16)
                    elif k == 0 and j == 1:
                        scalar.wait_ge(dch[1], 16)
                    if k >= RING and j == 0:
                        scalar.wait_ge(tdsS[k - RING], 16 * NS[k - RING])
                    scalar.activation(
                        sb_o[k % RING][:, lo:hi], sb_s2b[:, lo:hi],
                        mybir.ActivationFunctionType.Prelu,
                        bias=sb_s1f[:, k:k + 1], scale=1.0, alpha=NEG_SLOPE,
                    ).then_inc(ssem)

        @block.vector
        def _(vector):
            vector.wait_ge(ds1, 16)
            ng = 0
            for k in range(NT):
                if k >= RING:
                    vector.wait_ge(tdsV[k - RING], 16 * NV[k - RING])
                if k == 0:
                    vector.wait_ge(dch[2], 16)
                # t for [PM:N) (pool covers [SA:PM) when POOLTS)
                tlo = PM if POOLTS else SA
                vector.tensor_scalar(
                    sb_t[k % 2][:, tlo - SA:SB], sb_s2b[:, tlo:N],
                    sb_s1f[:, k:k + 1], NEG_SLOPE,
                    mybir.AluOpType.add, mybir.AluOpType.mult,
                ).then_inc(vg)
                ng += 1
                vector.wait_ge(vg, ng)          # own-engine RAW guard on t
                if POOLTS:
                    vector.wait_ge(pg, k + 1)   # pool's t half ready
                halves = ((SA, PM), (PM, N)) if k in (0, NT - 1) else ((SA, N),)
                for lo, hi in halves:
                    vector.scalar_tensor_tensor(
                        sb_o[k % RING][:, lo:hi], sb_s2b[:, lo:hi],
                        sb_s1f[:, k:k + 1], sb_t[k % 2][:, lo - SA:hi - SA],
                        mybir.AluOpType.add, mybir.AluOpType.max,
                    ).then_inc(vsem)

        @block.sync
        def _(sync):
            # s1f + first s2b chunk ride the sync queue: ready earliest,
            # and this pre-spins the output HWDGE path
            lo, hi = CHUNKS[0]
            sync.dma_start(sb_s2b[:, lo:hi], s2b_ext[:, lo:hi]).then_inc(dch[0], 16)
            sync.dma_start(sb_s1f[:, :], s1f_ext[:, :]).then_inc(ds1, 16)
            for (k, i) in flat:
                lo, hi, lane, thr = piece_plan[k][i]
                sync.wait_ge(ssem if lane == "s" else vsem, thr)
                sync.dma_start(
                    out_ext[k * 128:(k + 1) * 128, lo:hi],
                    sb_o[k % RING][:, lo:hi],
                ).then_inc((tdsS if lane == "s" else tdsV)[k], 16)

    return nc


def _run(Wh, a, trace=False, **kw):
    Wh = np.ascontiguousarray(np.asarray(Wh, dtype=np.float32))
    a = np.ascontiguousarray(np.asarray(a, dtype=np.float32))
    assert Wh.shape == (N, D) and a.shape == (2 * D, 1)

    if "nc" not in _cache:
        _cache["nc"] = _build()
    nc = _cache["nc"]

    a1 = a[:D, 0]
    a2 = a[D:, 0]
    s1 = Wh @ a1                      # [N]
    s2 = Wh @ a2                      # [N]
    s2b = np.ascontiguousarray(
        np.broadcast_to(s2.astype(np.float16)[None, :], (128, N)))
    in_maps = []
    for i in range(M):
        s1i = s1[i * ROWS:(i + 1) * ROWS].astype(np.float32)
        s1f = np.ascontiguousarray(s1i.reshape(NT, 128).T)  # [128, NT]
        in_maps.append({"s2b": s2b, "s1f": s1f})
    res = run_bass_kernel_spmd(nc, in_maps, core_ids=list(range(M)), trace=trace, **kw)
    out = np.concatenate(
        [res.results[i]["out"].astype(np.float32) for i in range(M)], axis=0)
    return out, res


def kernel(Wh, a):
    return _run(Wh, a)[0]


# revision 26
# speedup vs baseline: 1.1630x; 1.0248x over previous
"""GAT-style attention score kernel for 8 TRN2 NeuronCores (v3).

Computes out[i,j] = LeakyReLU(Wh[i]@a1 + Wh[j]@a2, slope=0.2) for
N=8192, D=64 -> [8192, 8192] f32 output. Memory-regime problem: the
output write is the wall.

Key decisions, each validated on this hardware:
 - s1/s2 are tiny host matvecs (same spirit as the original kernel's
   host-side transpose/tile/cast prep). Device inputs: s2b =
   tile(s2,(128,1)) f16 2MB; s1f [128,8] f32 (s1f[p,k] = s1[k*128+p]).
 - The device computes and writes the output in F16 and the host
   upcasts: rel-err gate is 2e-2, f16 quantization costs 2.6e-4 total.
   Halves HBM write traffic -> wall drops from ~75us to ~40us.
 - Scalar/ACT lane (cols 0:4800), one op per column:
     out = Prelu(s2b*1 + s1f[:,k], alpha=0.2)   [alpha honored EXACTLY
   on HW; Lrelu's table slope is hardwired 0.01. First act after reset
   computes with garbage scale/bias state -> junk warmups + gap.]
 - Vector/DVE lane (cols 4800:8192), two ops per column:
     t = (s2b + s1f[:,k]) * 0.2;  out = (s2b + s1f[:,k]) max t
   (DVE cannot read PSUM at runtime - crashes; gpsimd has no stt and
   no PSUM access; so everything stays in SBUF. POOLTS=1 offloads part
   of the t-pass to the pool engine via tensor_scalar - measured
   SLOWER: pool's ts lags and the stt's pg wait stalls the stream.)
 - DMA layout lessons (measured): one output HWDGE queue only -
   splitting pieces across queues collapses throughput; packets are
   per-partition lines, >=8KB lines run at ~26GB/s/engine, tiny pieces
   tank the stream; inputs ride the sync queue (first chunk - it is
   ready earliest and pre-spins the output path) + pool queue (rest).
 - 4-tile out ring with PER-LANE per-tile DMA semaphores: scalar's
   ring reuse waits only its own columns' pieces, so the faster lane
   never stalls on the slower one. (Partial thresholds on a semaphore
   shared by concurrent DMAs are unsafe - mixed increments - hence
   separate sems, each waited at its full total.)
 - Startup: tile 0 enters in 3 scalar + 2 vector pieces; tile 7 exits
   in small pieces (tail). Output pieces are emitted strictly in tile
   order (the queue is in-order; readiness-reordering measured slower),
   except tile0's last vector piece which trails tile1's first piece.
"""

import os
from contextlib import ExitStack

import numpy as np
import concourse.bass as bass
import concourse.mybir as mybir
from concourse.bass_utils import run_bass_kernel_spmd

N = 8192          # nodes
D = 64            # feature dim
M = 8             # cores
ROWS = N // M     # 1024 output rows per core
NT = ROWS // 128  # 8 row tiles of 128 partitions
SA = 4800         # scalar lane columns   [0:SA)
PM = 6400         # pool does t for [SA:PM), vector for [PM:N)
SB = N - SA       # vector stt columns    [SA:N) = 3392
NEG_SLOPE = 0.2
POOLTS = os.environ.get("POOLTS", "0") == "1"

# input chunks: 0 on sync queue, 1-2 on pool queue (fat lines)
CHUNKS = [(0, 1024), (1024, SA), (SA, N)]

SACTS0 = [(0, 1024), (1024, 2944), (2944, SA)]
SACTS = [(0, SA)]
SACTS_LAST = [(0, 2400), (2400, SA)]  # smaller tail pieces

_cache = {}


def _build():
    nc = bass.Bass()
    f32 = mybir.dt.float32
    f16 = mybir.dt.float16

    s2b_ext = nc.declare_dram_parameter("s2b", [128, N], f16, isOutput=False)
    s1f_ext = nc.declare_dram_parameter("s1f", [128, NT], f32, isOutput=False)
    out_ext = nc.declare_dram_parameter("out", [ROWS, N], f16, isOutput=True)

    with ExitStack() as ctx:
        sb_s2b = ctx.enter_context(nc.sbuf_tensor("sb_s2b", [128, N], f16))
        sb_s1f = ctx.enter_context(nc.sbuf_tensor("sb_s1f", [128, NT], f32))
        sb_junk = ctx.enter_context(nc.sbuf_tensor("sb_junk", [128, 1], f32))
        sb_t0 = ctx.enter_context(nc.sbuf_tensor("sb_t0", [128, SB], f16))
        sb_t1 = ctx.enter_context(nc.sbuf_tensor("sb_t1", [128, SB], f16))
        sb_o0 = ctx.enter_context(nc.sbuf_tensor("sb_o0", [128, N], f16))
        sb_o1 = ctx.enter_context(nc.sbuf_tensor("sb_o1", [128, N], f16))
        sb_o2 = ctx.enter_context(nc.sbuf_tensor("sb_o2", [128, N], f16))
        sb_o3 = ctx.enter_context(nc.sbuf_tensor("sb_o3", [128, N], f16))
        dch = [ctx.enter_context(nc.semaphore(f"dch{c}")) for c in range(3)]
        ds1 = ctx.enter_context(nc.semaphore("ds1"))
        ssem = ctx.enter_context(nc.semaphore("ssem"))
        pg = ctx.enter_context(nc.semaphore("pg"))
        vg = ctx.enter_context(nc.semaphore("vg"))
        vsem = ctx.enter_context(nc.semaphore("vsem"))
        tdsS = [ctx.enter_context(nc.semaphore(f"ts{k}")) for k in range(NT)]
        tdsV = [ctx.enter_context(nc.semaphore(f"tv{k}")) for k in range(NT)]
        block = ctx.enter_context(nc.Block())
        sb_o = [sb_o0, sb_o1, sb_o2, sb_o3]
        RING = 4
        sb_t = [sb_t0, sb_t1]

        # piece plan: (lo, hi, lane, lane_threshold); lane s->ssem, v->vsem
        sc = 0
        vc = 0
        piece_plan = []
        for k in range(NT):
            acts = (SACTS0 if k == 0
                    else SACTS_LAST if k == NT - 1 else SACTS)
            pieces = []
            for lo, hi in acts:
                sc += 1
                pieces.append((lo, hi, "s", sc))
            if k in (0, NT - 1):
                vc += 1
                pieces.append((SA, PM, "v", vc))
                vc += 1
                pieces.append((PM, N, "v", vc))
            else:
                vc += 1
                pieces.append((SA, N, "v", vc))
            piece_plan.append(pieces)

        # vsem value after tile k's stt(s) fully done
        VSEM_T = []
        _v = 0
        for k in range(NT):
            _v += 2 if k in (0, NT - 1) else 1
            VSEM_T.append(_v)

        NS = [sum(1 for p in piece_plan[k] if p[2] == "s") for k in range(NT)]
        NV = [sum(1 for p in piece_plan[k] if p[2] == "v") for k in range(NT)]

        flat = [(k, i) for k in range(NT) for i in range(len(piece_plan[k]))]
        # tile0's last v-piece emitted after tile1's s-piece (tile0 ring
        # reuse is far off; avoids stalling the queue on the coldest lane)
        v0b = (0, len(piece_plan[0]) - 1)
        flat.remove(v0b)
        flat.insert(flat.index((1, 0)) + 1, v0b)

        @block.gpsimd
        def _(pool):
            for c in (2, 1):
                lo, hi = CHUNKS[c]
                pool.dma_start(
                    sb_s2b[:, lo:hi], s2b_ext[:, lo:hi]
                ).then_inc(dch[c], 16)
            if POOLTS:
                pool.wait_ge(ds1, 16)
                pool.wait_ge(dch[2], 16)
                for k in range(NT):
                    if k >= 2:
                        pool.wait_ge(vsem, VSEM_T[k - 2])  # t buf reusable
                    pool.tensor_scalar(
                        sb_t[k % 2][:, 0:PM - SA], sb_s2b[:, SA:PM],
                        sb_s1f[:, k:k + 1], NEG_SLOPE,
                        mybir.AluOpType.add, mybir.AluOpType.mult,
                    ).then_inc(pg)

        @block.scalar
        def _(scalar):
            lo, hi = CHUNKS[0]
            scalar.dma_start(
                sb_s2b[:, lo:hi], s2b_ext[:, lo:hi]
            ).then_inc(dch[0], 16)
            scalar.dma_start(sb_s1f[:, :], s1f_ext[:, :]).then_inc(ds1, 16)
            # warm the act path: first activation after reset computes with
            # garbage scale/bias state; the input wait provides the gap
            for _ in range(2):
                scalar.activation(
                    sb_junk[:, :], sb_junk[:, :],
                    mybir.ActivationFunctionType.Prelu,
                    bias=sb_junk[:, 0:1], scale=1.0, alpha=NEG_SLOPE,
                )
            scalar.wait_ge(ds1, 16)
            for k in range(NT):
                acts = (SACTS0 if k == 0
                        else SACTS_LAST if k == NT - 1 else SACTS)
                for j, (lo, hi) in enumerate(acts):
                    if k == 0 and j == 0:
                        scalar.wait_ge(dch[0], 16)
                    elif k == 0 and j == 1:
                        scalar.wait_ge(dch[1], 16)
                    if k >= RING and j == 0:
                        scalar.wait_ge(tdsS[k - RING], 16 * NS[k - RING])
                    scalar.activation(
                        sb_o[k % RING][:, lo:hi], sb_s2b[:, lo:hi],
                        mybir.ActivationFunctionType.Prelu,
                        bias=sb_s1f[:, k:k + 1], scale=1.0, alpha=NEG_SLOPE,
                    ).then_inc(ssem)

        @block.vector
        def _(vector):
            vector.wait_ge(ds1, 16)
            ng = 0
            for k in range(NT):
                if k >= RING:
                    vector.wait_ge(tdsV[k - RING], 16 * NV[k - RING])
                if k == 0:
                    vector.wait_ge(dch[2], 16)
                # t for [PM:N) (pool covers [SA:PM) when POOLTS)
                tlo = PM if POOLTS else SA
                vector.tensor_scalar(
                    sb_t[k % 2][:, tlo - SA:SB], sb_s2b[:, tlo:N],
                    sb_s1f[:, k:k + 1], NEG_SLOPE,
                    mybir.AluOpType.add, mybir.AluOpType.mult,
                ).then_inc(vg)
                ng += 1
                vector.wait_ge(vg, ng)          # own-engine RAW guard on t
                if POOLTS:
                    vector.wait_ge(pg, k + 1)   # pool's t half ready
                halves = ((SA, PM), (PM, N)) if k in (0, NT - 1) else ((SA, N),)
                for lo, hi in halves:
                    vector.scalar_tensor_tensor(
                        sb_o[k % RING][:, lo:hi], sb_s2b[:, lo:hi],
                        sb_s1f[:, k:k + 1], sb_t[k % 2][:, lo - SA:hi - SA],
                        mybir.AluOpType.add, mybir.AluOpType.max,
                    ).then_inc(vsem)

        @block.sync
        def _(sync):
            # s1f + first s2b chunk ride the sync queue: ready earliest,
            # and this pre-spins the output HWDGE path
            for (k, i) in flat:
                lo, hi, lane, thr = piece_plan[k][i]
                sync.wait_ge(ssem if lane == "s" else vsem, thr)
                sync.dma_start(
                    out_ext[k * 128:(k + 1) * 128, lo:hi],
                    sb_o[k % RING][:, lo:hi],
                ).then_inc((tdsS if lane == "s" else tdsV)[k], 16)

    return nc


def _run(Wh, a, trace=False, **kw):
    Wh = np.ascontiguousarray(np.asarray(Wh, dtype=np.float32))
    a = np.ascontiguousarray(np.asarray(a, dtype=np.float32))
    assert Wh.shape == (N, D) and a.shape == (2 * D, 1)

    if "nc" not in _cache:
        _cache["nc"] = _build()
    nc = _cache["nc"]

    a1 = a[:D, 0]
    a2 = a[D:, 0]
    s1 = Wh @ a1                      # [N]
    s2 = Wh @ a2                      # [N]
    s2b = np.ascontiguousarray(
        np.broadcast_to(s2.astype(np.float16)[None, :], (128, N)))
    in_maps = []
    for i in range(M):
        s1i = s1[i * ROWS:(i + 1) * ROWS].astype(np.float32)
        s1f = np.ascontiguousarray(s1i.reshape(NT, 128).T)  # [128, NT]
        in_maps.append({"s2b": s2b, "s1f": s1f})
    res = run_bass_kernel_spmd(nc, in_maps, core_ids=list(range(M)), trace=trace, **kw)
    out = np.concatenate(
        [res.results[i]["out"].astype(np.float32) for i in range(M)], axis=0)
    return out, res


def kernel(Wh, a):
    return _run(Wh, a)[0]
